# revision 5
# baseline (speedup 1.0000x reference)
# Multi-head causal attention (B=4, S=2048, D=1024, H=16) on 8 TRN2 NeuronCores.
#
# Sharding: batch x query-chunk. Core c handles batch b=c//2 and two 512-row
# query chunks of that batch: cores with c%2==0 take real chunks (0, 3),
# c%2==1 take (1, 2). The SPMD program is identical on every core: it
# processes two query "slots" with fixed kk-tile capacities (8, 16); real
# chunk needs (4,8,12,16 tiles) are mapped into those capacities and the
# excess key tiles are zeroed by per-core causal-mask input data. Each core
# computes K/V projections for its whole batch (duplicated across the 2 cores
# sharing a batch) so no cross-core collectives are needed.
#
# All matmuls run as float32r (FP22-truncated fp32; full PE rate at moving
# dim >= 256). Attention uses the transposed-scores layout St[kk, q]:
#   Kt[d, s], Qt[d, q]; St = Kt_tile.T @ Qt_chunk   (2 heads row-packed)
#   P = exp(St) * mask
#   OT[dv, q] += V_aug[kk, 65].T @ P   -- V carries a ones column, so PSUM
#     row 64 accumulates the softmax denominators for free.
#   OT_norm = OT * broadcast(1/denom);  y[q, :] = sum_dc OT.T @ woT + b_o.
import sys

if '/opt/trn_rl_repo' not in sys.path:
    sys.path.insert(0, '/opt/trn_rl_repo')

import numpy as np

B, S, D = 4, 2048, 1024
H, DK = 16, 64
NCORES = 8
SC = 512
NKT = S // 128            # 16 kk tiles
HPN = D // 128            # 8 head-pairs
CAPS = (8, 16)            # kk-tile capacity per slot (uniform across cores)
CHUNKS = [(0, 3), (1, 2)]  # real chunk pair per core parity

_CACHE = {}


def _build_program():
    import contextlib

    import concourse.tile as tile
    from concourse import bacc, mybir

    F32 = mybir.dt.float32
    F32R = mybir.dt.float32r
    EXP = mybir.ActivationFunctionType.Exp

    nc = bacc.Bacc("TRN2", target_bir_lowering=False, debug=False,
                   num_devices=NCORES)

    xT_d = nc.dram_tensor("xT", [D, S], F32, kind="ExternalInput")
    xQT_d = nc.dram_tensor("xQT", [D, 2 * SC], F32, kind="ExternalInput")
    wqT_d = nc.dram_tensor("wqT", [D, D], F32, kind="ExternalInput")
    wkT_d = nc.dram_tensor("wkT", [D, D], F32, kind="ExternalInput")
    wvT_d = nc.dram_tensor("wvT", [D, D], F32, kind="ExternalInput")
    woT_d = nc.dram_tensor("woT", [D, D], F32, kind="ExternalInput")
    bias_d = nc.dram_tensor("bias", [1, D], F32, kind="ExternalInput")
    masks_d = nc.dram_tensor("masks", [128, NKT * SC], F32,
                             kind="ExternalInput")
    y_d = nc.dram_tensor("y", [2 * SC, D], F32, kind="ExternalOutput")

    with tile.TileContext(nc) as tc, contextlib.ExitStack() as ctx:
        smalls = ctx.enter_context(tc.tile_pool(name="smalls", bufs=1))
        p_rs = ctx.enter_context(tc.tile_pool(name="rs", bufs=2))
        p_bc = ctx.enter_context(tc.tile_pool(name="bc", bufs=2))
        p_OT = ctx.enter_context(tc.tile_pool(name="otp", bufs=1))
        p_dram = ctx.enter_context(
            tc.tile_pool(name="dram", bufs=1, space="DRAM"))

        masks_sb = smalls.tile([128, NKT * SC], F32, tag="masks")
        nc.sync.dma_start(masks_sb[:], masks_d.ap())
        bias_sb = smalls.tile([1, D], F32R, tag="bias")
        nc.sync.dma_start(bias_sb[:], bias_d.ap().bitcast(F32R))
        ones1f = smalls.tile([1, 128], F32, tag="ones1f")
        nc.vector.memset(ones1f[:], 1.0)
        ones1 = smalls.tile([1, 128], F32R, tag="ones1")
        nc.vector.tensor_copy(ones1[:], ones1f[:])
        ones16f = smalls.tile([128, 16], F32, tag="ones16f")
        nc.vector.memset(ones16f[:], 1.0)
        ones16 = smalls.tile([128, 16], F32R, tag="ones16")
        nc.vector.tensor_copy(ones16[:], ones16f[:])

        OT = p_OT.tile([128, HPN * 2 * SC], F32R, tag="OT")

        vaug = p_dram.tile([128, NKT * H * 65], F32R, tag="vaug")
        Ktd = p_dram.tile([128, HPN * S], F32R, tag="ktd")
        Qtd = p_dram.tile([128, HPN * 2 * SC], F32R, tag="qtd")

        # ---------------- V and K projections (use xT) ----------------
        with tc.tile_pool(name="xt", bufs=1) as p_xT, \
             tc.tile_pool(name="wf", bufs=1) as p_w, \
             tc.tile_pool(name="pb", bufs=4) as p_b, \
             tc.tile_pool(name="psp", bufs=6, space="PSUM") as psp:

            xT = p_xT.tile([128, 8 * S], F32R, tag="xT")
            for k in range(8):
                nc.sync.dma_start(
                    xT[:, k * S:(k + 1) * S],
                    xT_d.ap()[k * 128:(k + 1) * 128, :].bitcast(F32R))

            # V = x @ wv.T in natural [s, dv] layout -> vaug (+ ones column)
            wv = p_w.tile([128, 8 * D], F32R, tag="w")
            for k in range(8):
                nc.sync.dma_start(
                    wv[:, k * D:(k + 1) * D],
                    wvT_d.ap()[k * 128:(k + 1) * 128, :].bitcast(F32R))
            for st in range(NKT):
                nc.sync.dma_start(
                    vaug[:, st * 1040:(st + 1) * 1040]
                    .rearrange("p (h c) -> p h c", c=65)[:, :, 64:65],
                    ones16[:].rearrange("p (h c) -> p h c", c=1))
                for dvc in range(2):
                    ps = psp.tile([128, 512], F32, tag="ps")
                    for k in range(8):
                        nc.tensor.matmul(
                            ps[:],
                            xT[:, k * S + st * 128:k * S + (st + 1) * 128],
                            wv[:, k * D + dvc * 512:k * D + (dvc + 1) * 512],
                            start=(k == 0), stop=(k == 7))
                    vb = p_b.tile([128, 512], F32R, tag="vb")
                    nc.vector.tensor_copy(vb[:], ps[:])
                    off = st * 1040 + dvc * 520
                    nc.sync.dma_start(
                        vaug[:, off:off + 520]
                        .rearrange("p (h c) -> p h c", c=65)[:, :, 0:64],
                        vb[:].rearrange("p (h c) -> p h c", c=64))

            # Kt[dout, s] -> Ktd
            wk = p_w.tile([128, 8 * D], F32R, tag="w")
            for k in range(8):
                nc.sync.dma_start(
                    wk[:, k * D:(k + 1) * D],
                    wkT_d.ap()[k * 128:(k + 1) * 128, :].bitcast(F32R))
            for hp in range(HPN):
                for sc in range(4):
                    ps = psp.tile([128, 512], F32, tag="ps")
                    for k in range(8):
                        nc.tensor.matmul(
                            ps[:],
                            wk[:, k * D + hp * 128:k * D + (hp + 1) * 128],
                            xT[:, k * S + sc * 512:k * S + (sc + 1) * 512],
                            start=(k == 0), stop=(k == 7))
                    kb = p_b.tile([128, 512], F32R, tag="vb")
                    nc.vector.tensor_copy(kb[:], ps[:])
                    nc.sync.dma_start(
                        Ktd[:, hp * S + sc * 512:hp * S + (sc + 1) * 512],
                        kb[:])

        # ---------------- Q projection (uses pre-gathered xQT) ----------
        with tc.tile_pool(name="xq", bufs=1) as p_xq, \
             tc.tile_pool(name="wf2", bufs=1) as p_w2, \
             tc.tile_pool(name="pb2", bufs=4) as p_b2, \
             tc.tile_pool(name="psq", bufs=6, space="PSUM") as psq:
            xQT = p_xq.tile([128, 8 * 2 * SC], F32R, tag="xQT")
            for k in range(8):
                nc.sync.dma_start(
                    xQT[:, k * 2 * SC:(k + 1) * 2 * SC],
                    xQT_d.ap()[k * 128:(k + 1) * 128, :].bitcast(F32R))
            wq = p_w2.tile([128, 8 * D], F32R, tag="w2")
            for k in range(8):
                nc.sync.dma_start(
                    wq[:, k * D:(k + 1) * D],
                    wqT_d.ap()[k * 128:(k + 1) * 128, :].bitcast(F32R))
            for hp in range(HPN):
                for ci in range(2):
                    ps = psq.tile([128, 512], F32, tag="ps")
                    for k in range(8):
                        nc.tensor.matmul(
                            ps[:],
                            wq[:, k * D + hp * 128:k * D + (hp + 1) * 128],
                            xQT[:, k * 2 * SC + ci * SC:
                                k * 2 * SC + (ci + 1) * SC],
                            start=(k == 0), stop=(k == 7))
                    qb = p_b2.tile([128, 512], F32R, tag="qb")
                    nc.vector.tensor_copy(qb[:], ps[:])
                    nc.sync.dma_start(
                        Qtd[:, hp * 2 * SC + ci * SC:
                            hp * 2 * SC + (ci + 1) * SC], qb[:])

        # ---------------- attention ----------------
        with tc.tile_pool(name="kts", bufs=6) as p_kt, \
             tc.tile_pool(name="qts", bufs=3) as p_qt, \
             tc.tile_pool(name="vts", bufs=6) as p_vt, \
             tc.tile_pool(name="pp", bufs=10) as p_P, \
             tc.tile_pool(name="pst", bufs=4, space="PSUM") as p_st, \
             tc.tile_pool(name="pav", bufs=4, space="PSUM") as p_av:

            Ktv = Ktd[:].rearrange("p (hp s) -> p hp s", hp=HPN)
            Qtv = Qtd[:].rearrange("p (hp q) -> p hp q", hp=HPN)

            for ci, cap in enumerate(CAPS):
                for bl in range(HPN // 2):
                    qt = p_qt.tile([128, 2 * SC], F32R, tag="qt")
                    nc.sync.dma_start(
                        qt[:].rearrange("p (a q) -> p a q", a=2),
                        Qtv[:, 2 * bl:2 * bl + 2, ci * SC:(ci + 1) * SC])
                    av = [p_av.tile([128, 512], F32, tag="av",
                                    name=f"av_{ci}_{bl}_{i}")
                          for i in range(4)]
                    for t in range(cap):
                        kt = p_kt.tile([128, 256], F32R, tag="kt")
                        nc.sync.dma_start(
                            kt[:].rearrange("p (a s) -> p a s", a=2),
                            Ktv[:, 2 * bl:2 * bl + 2, t * 128:(t + 1) * 128])
                        vt = p_vt.tile([128, 260], F32R, tag="vt")
                        nc.sync.dma_start(
                            vt[:],
                            vaug[:, t * 1040 + bl * 260:
                                 t * 1040 + (bl + 1) * 260])
                        for hi in range(4):
                            r0 = 64 * (hi % 2)
                            cb = 128 * (hi // 2)
                            st = p_st.tile([128, 512], F32, tag="st")
                            nc.tensor.matmul(
                                st[:],
                                kt[r0:r0 + 64, cb:cb + 128],
                                qt[r0:r0 + 64,
                                   (hi // 2) * SC:(hi // 2 + 1) * SC],
                                start=True, stop=True,
                                tile_position=(r0, 0))
                            p1 = p_P.tile([128, 512], F32R, tag="p")
                            nc.scalar.activation(p1[:], st[:], EXP)
                            if ci == 0 or t >= 8:
                                p2 = p_P.tile([128, 512], F32R, tag="p")
                                nc.vector.tensor_mul(
                                    p2[:], p1[:],
                                    masks_sb[:, t * SC:(t + 1) * SC])
                                p1 = p2
                            nc.tensor.matmul(
                                av[hi][0:65, :],
                                vt[:, hi * 65:hi * 65 + 65],
                                p1[:],
                                start=(t == 0), stop=(t == cap - 1))
                    rs = p_rs.tile([1, 2048], F32, tag="rs")
                    for hi in range(4):
                        nc.vector.reciprocal(
                            rs[0:1, hi * 512:hi * 512 + 512],
                            av[hi][64:65, :])
                    bc = p_bc.tile([128, 2048], F32, tag="bc")
                    nc.gpsimd.partition_broadcast(bc[:], rs[:])
                    for hi in range(4):
                        r0 = 64 * (hi % 2)
                        hp = 2 * bl + hi // 2
                        nc.vector.tensor_mul(
                            OT[r0:r0 + 64,
                               hp * 2 * SC + ci * SC:hp * 2 * SC + (ci + 1) * SC],
                            av[hi][0:64, :],
                            bc[r0:r0 + 64, hi * 512:hi * 512 + 512])

        # ---------------- output projection ----------------
        with tc.tile_pool(name="wo", bufs=1) as p_wo, \
             tc.tile_pool(name="ybp", bufs=4) as p_yb, \
             tc.tile_pool(name="psy", bufs=6, space="PSUM") as psy:
            wo = p_wo.tile([128, 8 * D], F32R, tag="wo")
            for k in range(8):
                nc.sync.dma_start(
                    wo[:, k * D:(k + 1) * D],
                    woT_d.ap()[k * 128:(k + 1) * 128, :].bitcast(F32R))
            for qi in range(8):
                for nc2 in range(2):
                    ps = psy.tile([128, 512], F32, tag="psy")
                    for dc in range(8):
                        nc.tensor.matmul(
                            ps[:],
                            OT[:, dc * 2 * SC + qi * 128:
                               dc * 2 * SC + (qi + 1) * 128],
                            wo[:, dc * D + nc2 * 512:dc * D + (nc2 + 1) * 512],
                            start=(dc == 0), stop=False)
                    nc.tensor.matmul(
                        ps[:], ones1[:],
                        bias_sb[0:1, nc2 * 512:(nc2 + 1) * 512],
                        start=False, stop=True)
                    yb = p_yb.tile([128, 512], F32, tag="yb")
                    nc.vector.tensor_copy(yb[:], ps[:])
                    nc.sync.dma_start(
                        y_d.ap()[qi * 128:(qi + 1) * 128,
                                 nc2 * 512:(nc2 + 1) * 512], yb[:])

    nc.compile()
    return nc


def _get_program():
    if 'nc' not in _CACHE:
        _CACHE['nc'] = _build_program()
    return _CACHE['nc']


def _tri_masks():
    # tri[r][p, f] = 1.0 if p <= f - 128*r else 0  (diagonal kk-tile r of a
    # 512-wide query chunk)
    p = np.arange(128)[:, None]
    f = np.arange(SC)[None, :]
    return [(p <= f - 128 * r).astype(np.float32) for r in range(4)]


def _masks_for_core(c):
    tri = _tri_masks()
    ones = np.ones((128, SC), np.float32)
    zeros = np.zeros((128, SC), np.float32)
    j_pair = CHUNKS[c % 2]
    out = np.zeros((128, NKT * SC), np.float32)
    for ci, cap in enumerate(CAPS):
        j = j_pair[ci]
        t0 = 0 if ci == 0 else 8
        for t in range(t0, cap):
            if t < 4 * j:
                m = ones
            elif t < 4 * j + 4:
                m = tri[t - 4 * j]
            else:
                m = zeros
            out[:, t * SC:(t + 1) * SC] = m
    return out


def kernel(x, w_q, w_k, w_v, w_o, b_o):
    from concourse.bass_utils import run_bass_kernel_spmd

    x = np.asarray(x, dtype=np.float32)
    nc = _get_program()

    scale = np.float32(1.0 / np.sqrt(DK))
    common = {
        "wqT": np.ascontiguousarray(np.asarray(w_q, np.float32).T * scale),
        "wkT": np.ascontiguousarray(np.asarray(w_k, np.float32).T),
        "wvT": np.ascontiguousarray(np.asarray(w_v, np.float32).T),
        "woT": np.ascontiguousarray(np.asarray(w_o, np.float32).T),
        "bias": np.ascontiguousarray(np.asarray(b_o, np.float32)[None, :]),
    }

    in_maps = []
    for c in range(NCORES):
        b = c // 2
        j1, j2 = CHUNKS[c % 2]
        xb = x[b]
        xq = np.concatenate(
            [xb[j1 * SC:(j1 + 1) * SC], xb[j2 * SC:(j2 + 1) * SC]], axis=0)
        in_maps.append({
            "xT": np.ascontiguousarray(xb.T),
            "xQT": np.ascontiguousarray(xq.T),
            "masks": _masks_for_core(c),
            **common,
        })

    res = run_bass_kernel_spmd(nc, in_maps, core_ids=list(range(NCORES)),
                               trace=_CACHE.get('trace', False),
                               tmpdir=_CACHE.get('tmpdir'))
    _CACHE['last_res'] = res

    y = np.empty((B, S, D), dtype=np.float32)
    for c in range(NCORES):
        b = c // 2
        j1, j2 = CHUNKS[c % 2]
        yc = res.results[c]["y"]
        y[b, j1 * SC:(j1 + 1) * SC] = yc[0:SC]
        y[b, j2 * SC:(j2 + 1) * SC] = yc[SC:2 * SC]
    return y
